# revision 1
# baseline (speedup 1.0000x reference)
"""Trainium2 Bass kernel for the grouped TF->gene sparse decoder (AEDecoder).

Math (reference):
  h1 = leaky_relu(features[:,:,None] * w1 + b1)            # [B,T,K]
  h2 = leaky_relu(einsum('btj,tjk->btk', h1, w2) + b2)     # [B,T,K]
  out = einsum('bgek,gek->bg', h2[:, edge_tf, :], w3) + b3 # [B,G]

Device formulation (per core, raw-Bass Block with explicit semaphores):
  - Everything transposed: contract dim c=(t,k) = 2048 rows = 16 chunks of
    128 partitions; batch on the free axis.
  - h1T_c = Prelu(featT_c * w1_c + b1_c)           (ACT, per-partition scale/bias)
  - h2T_c = Prelu(W2blk_c^T @ h1T_c + b2_c)        (PE block-diag 4x4 + ACT)
  - out[b, g] = sum_c h2T[c, b]*S[c, g] + b3[g]    (PE dense bf16 matmul)
    S is the host-packed scatter of w3 over (t,k) rows: S[4t+k, g] += w3[g,e,k]
    for each edge e with edge_tf[g,e]==t. Sparsity (12 nnz/col) is not
    exploitable on the PE (random TF spread), so the contraction runs dense.
  - 20 output tiles [128b x 512g] pipelined through a 7-bank PSUM ring with
    a bank-aware staggered schedule (tiles 0-4 open immediately on the pm
    banks, 5-6 after the h2-build vacates the ph banks, refills open as
    banks free up); the h2-build itself is interleaved one chunk per
    schedule step so the main contraction starts ~5 us earlier; DVE adds b3
    on eviction (stop-emission order); per-tile out DMA.
  - S (10.5 MB bf16/core) streams in 8 DMAs with per-piece semaphores so the
    PE starts after ~1/8 of S has landed; warmup matmuls keep the PE HAM
    activity window fed (else it drops to the 1.2 GHz p-state during the
    DMA-paced phase).

Sharding: 8 cores = 8 gene-groups (2500 genes each), full batch per core.
S / b3 differ per core; features (k-replicated, transposed) and w1/b1/w2/b2
are replicated. Host does layout/index packing only; all FLOPs on device.
"""

import os

import numpy as np
import ml_dtypes

import concourse.bass as bass
import concourse.mybir as mybir
import concourse.tile as tile
from concourse.bass_utils import run_bass_kernel_spmd

BF16 = mybir.dt.bfloat16
F32 = mybir.dt.float32
AFT = mybir.ActivationFunctionType

B, T, K, G, EPG = 512, 512, 4, 20000, 3
NCORES = 8
BGRP, GGRP = 1, 8            # batch-groups x gene-groups
BSH = B // BGRP              # 512 cells per core (full batch)
GSH = G // GGRP              # 2500 genes per core
NCH = (T * K) // 128         # 16 contract chunks
GT = 512                     # gene tile (matmul free dim)
GP = ((GSH + GT - 1) // GT) * GT   # 10240 padded genes
NGT = GP // GT               # 20 gene tiles

ALPHA = 0.01                 # leaky_relu slope

_GRAPH = None
LAST_RESULT = None
_LDW_PATCHED = False


def _enable_ldw_opt():
    """Flip walrus --enable-ldw-opt to true: elides redundant LDWEIGHTS for
    back-to-back matmuls that share the same stationary operand (our inner
    gene-tile loop reuses one h2 block across NGT matmuls)."""
    global _LDW_PATCHED
    if _LDW_PATCHED or not os.environ.get("KERNEL_LDW_OPT"):
        return
    import concourse.bass_utils as bu
    orig = bu.run_command

    def _run(cmd, **kw):
        cmd = ["--enable-ldw-opt=true" if c == "--enable-ldw-opt=false" else c
               for c in cmd]
        return orig(cmd, **kw)

    bu.run_command = _run
    _LDW_PATCHED = True


def _ensure_profile_hook():
    """Register an NTFF profile hook when the image lacks antenv.axon_hooks.

    Replicates trn_agent_boot's ctypes shim against libaxon_pjrt.so so
    run_bass_kernel_spmd(trace=True) can capture exec_time_ns.
    """
    import contextlib
    import ctypes
    import sys
    import types

    try:
        import antenv.axon_hooks  # noqa: F401
        return
    except ImportError:
        pass

    holder = {}
    mod = types.ModuleType("antenv.axon_hooks")
    mod.set_axon_ntff_profile_hook = lambda h: holder.__setitem__("h", h)
    mod.get_axon_ntff_profile_hook = lambda: holder.get("h")
    sys.modules["antenv.axon_hooks"] = mod

    so_path = "/opt/axon/libaxon_pjrt.so"
    try:
        lib = ctypes.CDLL(so_path)
    except OSError:
        return
    if not hasattr(lib, "axon_start_nrt_profile"):
        return
    lib.axon_start_nrt_profile.argtypes = [
        ctypes.POINTER(ctypes.c_int64), ctypes.c_size_t,
    ]
    lib.axon_start_nrt_profile.restype = ctypes.c_int64
    lib.axon_stop_nrt_profile.argtypes = [ctypes.c_char_p]
    lib.axon_stop_nrt_profile.restype = ctypes.c_int64

    @contextlib.contextmanager
    def _hook(output_dir, device_ids):
        import jax
        jax.devices()
        if device_ids:
            ids = (ctypes.c_int64 * len(device_ids))(*device_ids)
            rc = lib.axon_start_nrt_profile(ids, len(device_ids))
        else:
            rc = lib.axon_start_nrt_profile(None, 0)
        if rc != 0:
            raise RuntimeError(f"axon_start_nrt_profile rc={rc}")
        try:
            yield
        finally:
            n = lib.axon_stop_nrt_profile(str(output_dir).encode())
            print(f"profile: {n} ntff file(s) written to {output_dir}")

    holder["h"] = _hook

    import concourse.bass_utils as bu
    bu.upload_artifacts = lambda tmpdir: tmpdir


def _build_graph():
    from contextlib import ExitStack

    nc = bass.Bass()

    featT_h = nc.declare_dram_parameter("featT", [128, NCH, BSH], BF16, isOutput=False)
    s_h = nc.declare_dram_parameter("S", [128, NCH, GP], BF16, isOutput=False)
    w2blk_h = nc.declare_dram_parameter("W2blk", [128, NCH, 128], BF16, isOutput=False)
    cols_h = nc.declare_dram_parameter("cols", [128, 3 * NCH], F32, isOutput=False)
    b3rep_h = nc.declare_dram_parameter("b3rep", [128, GP], F32, isOutput=False)
    out_h = nc.declare_dram_parameter("out", [B, GSH], F32, isOutput=True)

    NBT = BSH // 128          # 4 b-tiles
    NT = NGT * NBT            # 20 (gene-tile, b-tile) output tiles
    NPM = 5                   # main psum ring (one per gene tile)
    N_CONST_DMAS = 4          # cols, b3rep, featT, w2blk

    with ExitStack() as es:
        s_sb = es.enter_context(nc.sbuf_tensor("s_sb", [128, NCH, GP], BF16))
        featT = es.enter_context(nc.sbuf_tensor("ft_sb", [128, NCH, BSH], BF16))
        h1 = es.enter_context(nc.sbuf_tensor("h1_sb", [128, NCH, BSH], BF16))
        h2 = es.enter_context(nc.sbuf_tensor("h2_sb", [128, NCH, BSH], BF16))
        w2blk = es.enter_context(nc.sbuf_tensor("w2_sb", [128, NCH, 128], BF16))
        cols = es.enter_context(nc.sbuf_tensor("cols_sb", [128, 3 * NCH], F32))
        b3rep = es.enter_context(nc.sbuf_tensor("b3rep_sb", [128, GP], F32))
        ot = [es.enter_context(nc.sbuf_tensor(f"ot{j}", [128, GP], F32)) for j in range(NBT)]
        ph = [es.enter_context(nc.psum_tensor(f"ph{j}", [128, BSH], F32)) for j in range(2)]
        pm = [es.enter_context(nc.psum_tensor(f"pm{j}", [128, GT], F32)) for j in range(NPM)]
        pwarm = es.enter_context(nc.psum_tensor("pwarm", [128, GT], F32))

        w1c = cols[:, 0:NCH]
        b1c = cols[:, NCH : 2 * NCH]
        b2c = cols[:, 2 * NCH : 3 * NCH]

        class _Sched:  # shared schedule holder
            pass
        tc = _Sched()
        NT_ALL0 = (BSH // 128) * NGT
        _start = {0: 1, 1: 2, 2: 3, 3: 4, 4: 5, 5: NCH, 6: NCH + 1}
        _bank = {j: j for j in range(7)}
        _prev = {}
        _ten = list(range(7))
        _free = [_start[j] + NCH for j in range(7)]
        for _j in range(7, NT_ALL0):
            _b = min(range(7), key=lambda x: _free[x])
            _s = max(_free[_b] + 1, _start[_j - 1] + 2)
            _bank[_j] = _b
            _start[_j] = _s
            _prev[_j] = _ten[_b]
            _ten[_b] = _j
            _free[_b] = _s + NCH
        _eorder = sorted(range(NT_ALL0), key=lambda t: _start[t])
        _erank = {t: i for i, t in enumerate(_eorder)}
        tc.SCHED = (_start, _bank, _prev, _eorder, _erank)

        with (
            nc.Block() as block,
            nc.semaphore("consts") as sem_consts,
            nc.semaphore("sg0") as sg0,
            nc.semaphore("sg1") as sg1,
            nc.semaphore("sg2") as sg2,
            nc.semaphore("sg3") as sg3,
            nc.semaphore("act") as sem_act,
            nc.semaphore("peh") as sem_peh,
            nc.semaphore("pem") as sem_pem,
            nc.semaphore("ev") as sem_ev,
            nc.semaphore("od") as sem_od,
            nc.semaphore("b3") as sem_b3,
            nc.semaphore("ft0") as ft0,
            nc.semaphore("ft1") as ft1,
            nc.semaphore("ft2") as ft2,
            nc.semaphore("ft3") as ft3,
            nc.semaphore("sh0") as sh0,
            nc.semaphore("sh1") as sh1,
            nc.semaphore("sh2") as sh2,
            nc.semaphore("sh3") as sh3,
        ):
            sft = [ft0, ft1, ft2, ft3]
            shalf = [sh0, sh1, sh2, sh3]  # second half of each S group
            sgrp = [sg0, sg1, sg2, sg3]

            @block.sync
            def _(sync: bass.BassEngine):
                sync.dma_start(out=cols[:], in_=cols_h[:]).then_inc(sem_consts, 16)
                sync.dma_start(out=w2blk[:], in_=w2blk_h[:]).then_inc(sem_consts, 16)
                sync.dma_start(
                    out=featT[:, 0:4, :], in_=featT_h[:, 0:4, :]
                ).then_inc(ft0, 16)
                sync.dma_start(
                    out=s_sb[:, 0:2, :], in_=s_h[:, 0:2, :]
                ).then_inc(sgrp[0], 16)
                sync.dma_start(
                    out=s_sb[:, 2:4, :], in_=s_h[:, 2:4, :]
                ).then_inc(shalf[0], 16)
                for p in range(1, 4):
                    sync.dma_start(
                        out=featT[:, 4 * p : 4 * (p + 1), :],
                        in_=featT_h[:, 4 * p : 4 * (p + 1), :],
                    ).then_inc(sft[p], 16)
                for j in range(1, 4):
                    sync.dma_start(
                        out=s_sb[:, 4 * j : 4 * j + 2, :],
                        in_=s_h[:, 4 * j : 4 * j + 2, :],
                    ).then_inc(sgrp[j], 16)
                    sync.dma_start(
                        out=s_sb[:, 4 * j + 2 : 4 * j + 4, :],
                        in_=s_h[:, 4 * j + 2 : 4 * j + 4, :],
                    ).then_inc(shalf[j], 16)
                sync.dma_start(out=b3rep[:], in_=b3rep_h[:]).then_inc(sem_b3, 16)
                _, _, _, evict_order, _ = tc.SCHED
                for i, t in enumerate(evict_order):
                    m, n = t // NGT, t % NGT
                    w = min(GSH - n * GT, GT)
                    sync.wait_ge(sem_ev, i + 1)
                    sync.dma_start(
                        out=out_h[m * 128 : (m + 1) * 128, n * GT : n * GT + w],
                        in_=ot[m][:, n * GT : n * GT + w],
                    ).then_inc(sem_od, 16)
                sync.wait_ge(sem_od, 16 * NBT * NGT)

            @block.scalar
            def _(scalar: bass.BassEngine):
                scalar.wait_ge(sem_consts, 32)
                for c in range(NCH):
                    if c % 4 == 0:
                        scalar.wait_ge(sft[c // 4], 16)
                    scalar.activation(
                        h1[:, c, :], featT[:, c, :], AFT.Prelu,
                        bias=b1c[:, c : c + 1], scale=w1c[:, c : c + 1], alpha=ALPHA,
                    ).then_inc(sem_act)
                    scalar.wait_ge(sem_peh, c + 1)
                    scalar.activation(
                        h2[:, c, :], ph[c % 2][:], AFT.Prelu,
                        bias=b2c[:, c : c + 1], alpha=ALPHA,
                    ).then_inc(sem_act)

            @block.tensor
            def _(tensor: bass.BassEngine):
                def warm(k, n=BSH):
                    # keep the PE activity window fed so HAM ramps to full clock
                    for _ in range(k):
                        tensor.matmul(
                            pwarm[:, :n], featT[:, 0, 0:128], featT[:, 0, :n],
                            start=True, stop=True, skip_group_check=True,
                        )
                warm(16)  # spin from t=0 (garbage reads) so HAM ramps early
                pm7 = pm + ph
                NT_ALL = NBT * NGT
                start_step, bank_of, prev_tenant, evict_order, evict_rank = tc.SCHED

                sgrp_waited = [False] * 8
                act_waited = [False] * NCH
                h2_built = 0
                n_steps = max(start_step.values()) + NCH
                for s in range(n_steps):
                    if h2_built < NCH:
                        c = h2_built
                        if c >= 4:
                            warm(1, 128)  # bridge ACT latency, keep HAM fed
                        tensor.wait_ge(sem_act, 2 * c + 1)
                        tensor.matmul(
                            ph[c % 2][:], w2blk[:, c, :], h1[:, c, :],
                            start=True, stop=True,
                        ).then_inc(sem_peh)
                        if s == 0:
                            warm(3)
                        h2_built += 1
                    for t in range(NT_ALL):
                        ci = s - start_step[t]
                        if ci < 0 or ci >= NCH:
                            continue
                        m, n = t // NGT, t % NGT
                        w = min(GSH - n * GT, GT)
                        gsl = slice(n * GT, n * GT + w)
                        c = ci
                        if ci == 0:
                            if t in prev_tenant:
                                tensor.wait_ge(sem_ev, evict_rank[prev_tenant[t]] + 1)
                            if t in (5, 6):
                                # ph bank: wait for the h2-build epilogue on ACT
                                tensor.wait_ge(sem_act, 2 * NCH)
                        if not act_waited[c]:
                            tensor.wait_ge(sem_act, 2 * c + 2)
                            act_waited[c] = True
                        if not sgrp_waited[c // 2]:
                            if c < 4:
                                warm(8)
                            else:
                                warm(2, 128)
                            sem = sgrp[c // 4] if (c % 4) < 2 else shalf[c // 4]
                            tensor.wait_ge(sem, 16)
                            sgrp_waited[c // 2] = True
                        mm = tensor.matmul(
                            pm7[bank_of[t]][:, :w],
                            h2[:, c, m * 128 : (m + 1) * 128],
                            s_sb[:, c, gsl],
                            start=(ci == 0), stop=(ci == NCH - 1),
                            skip_group_check=True,
                        )
                        if ci == NCH - 1:
                            mm.then_inc(sem_pem)

            @block.vector
            def _(vector: bass.BassEngine):
                _, bank_of, _, evict_order, _ = tc.SCHED
                vector.wait_ge(sem_b3, 16)
                for i, t in enumerate(evict_order):
                    m, n = t // NGT, t % NGT
                    w = min(GSH - n * GT, GT)
                    vector.wait_ge(sem_pem, i + 1)
                    vector.tensor_add(
                        ot[m][:, n * GT : n * GT + w], (pm + ph)[bank_of[t]][:, :w],
                        b3rep[:, n * GT : n * GT + w],
                    ).then_inc(sem_ev)

    return nc


def _prep_inputs(features, w1, b1, w2, b2, w3, b3, edge_tf):
    """Host-side packing: layout/index preprocessing only."""
    bf = ml_dtypes.bfloat16
    featT = np.repeat(np.ascontiguousarray(features.T), K, axis=0)  # [2048, B]
    featT = np.ascontiguousarray(
        featT.reshape(NCH, 128, B).transpose(1, 0, 2)).astype(bf)  # [128, NCH, B]

    w1c = w1.reshape(T * K).reshape(NCH, 128).T.astype(np.float32)
    b1c = b1.reshape(T * K).reshape(NCH, 128).T.astype(np.float32)
    b2c = b2.reshape(T * K).reshape(NCH, 128).T.astype(np.float32)
    cols = np.concatenate([w1c, b1c, b2c], axis=1).copy()

    w2r = w2.reshape(NCH, 32, K, K)
    w2blk = np.zeros((NCH, 32, K, 32, K), np.float32)
    for i in range(32):
        w2blk[:, i, :, i, :] = w2r[:, i]
    w2blk = np.ascontiguousarray(
        w2blk.reshape(NCH, 128, 128).transpose(1, 0, 2)).astype(bf)

    s_gg, b3_gg = [], []
    for gg in range(GGRP):
        gsl = slice(gg * GSH, (gg + 1) * GSH)
        et = edge_tf[gsl]                      # [GSH, EPG]
        wv = w3[gsl].astype(np.float32)        # [GSH, EPG, K]
        s = np.zeros((T * K, GP), np.float32)
        rows = (et[:, :, None] * K + np.arange(K)[None, None, :])  # [GSH,EPG,K]
        scols = np.broadcast_to(np.arange(GSH)[:, None, None], rows.shape)
        np.add.at(s, (rows.ravel(), scols.ravel()), wv.ravel())
        s_gg.append(np.ascontiguousarray(
            s.reshape(NCH, 128, GP).transpose(1, 0, 2)).astype(bf))
        b3p = np.zeros((GP,), np.float32)
        b3p[:GSH] = b3[gsl]
        b3_gg.append(np.ascontiguousarray(np.broadcast_to(b3p, (128, GP))))

    in_maps = []
    for core in range(NCORES):
        gg = core
        in_maps.append({
            "featT": featT,
            "S": s_gg[gg],
            "W2blk": w2blk,
            "cols": cols,
            "b3rep": b3_gg[gg],
        })
    return in_maps


def kernel(features, w1, b1, w2, b2, w3, b3, edge_tf):
    global _GRAPH, LAST_RESULT
    features, w1, b1, w2, b2, w3, b3, edge_tf = (
        np.asarray(x) for x in (features, w1, b1, w2, b2, w3, b3, edge_tf)
    )
    if _GRAPH is None:
        _GRAPH = _build_graph()
    in_maps = _prep_inputs(features, w1, b1, w2, b2, w3, b3, edge_tf)
    trace = bool(int(os.environ.get("KERNEL_TRACE", "0")))
    if trace:
        _ensure_profile_hook()
    _enable_ldw_opt()
    res = run_bass_kernel_spmd(
        _GRAPH, in_maps, core_ids=list(range(NCORES)), trace=trace,
    )
    LAST_RESULT = res
    out = np.zeros((B, G), np.float32)
    for core in range(NCORES):
        out[:, core * GSH : (core + 1) * GSH] = (
            np.asarray(res.results[core]["out"]).astype(np.float32)
        )
    return out

